# revision 7
# baseline (speedup 1.0000x reference)
"""DiffVG-style circle renderer on 8 Trainium2 NeuronCores.

Strategy: shard the 1024x1024 image by rows (128 rows per core). Each core
composites only the circles whose vertical span intersects its row band
(sigmoid coverage is < 1.2e-7 beyond r+8 px of the edge), processing each
circle front-to-back on a 224-column window around its center:

    cov = sigmoid(2*r - 2*sqrt(dx^2 + dy^2))    (per pixel)
    w   = T * cov                                (T = transmittance, init 1)
    C  += w * (alpha * color);  T -= alpha * w   (premultiplied accumulation)

Final:  rgb = C, a = 1 - T  (identical to the sequential 'over' scan).

Engines: PE builds d^2 via a K=2 matmul (outer sum of dx^2 row and dy^2
column), ACT does batched sqrt then batched sigmoid (one table-set switch
total), DVE runs the w/T chain plus R,G MACs, GPSIMD the z-affine and the
B MAC. Dynamic per-circle windows use register-offset access patterns.
"""

import sys

if "/opt/trn_rl_repo" not in sys.path:
    sys.path.insert(0, "/opt/trn_rl_repo")

import numpy as np

import concourse.bass as bass
import concourse.bacc as bacc
import concourse.mybir as mybir
from concourse.tile import TileContext, add_dep_helper
from concourse import bass_utils

H = 1024
W = 1024
ROWS = 128          # rows per core
N_CORES = 8
WIN = 224           # column window per circle (covers 2*(r+8) for r < 100)
MARGIN = 8.0        # sigmoid(-2*8) ~ 1.1e-7
CHUNK = 72          # max circle slots compiled per phase-pass (SBUF budget)
F32 = mybir.dt.float32
AF = mybir.ActivationFunctionType
OP = mybir.AluOpType


def _build_core_inputs(centers, radii, colors, core):
    """Pack per-core, per-slot parameters (slots ordered top-circle-first)."""
    y0 = ROWS * core
    cy = centers[:, 1].astype(np.float64)
    cx = centers[:, 0].astype(np.float64)
    r = radii.astype(np.float64)
    keep = (cy + r + MARGIN >= y0 + 0.5) & (cy - r - MARGIN <= y0 + ROWS - 0.5)
    idx = np.where(keep)[0][::-1]  # reversed: topmost (last-drawn) first
    return idx, cx[idx], cy[idx], r[idx], colors[idx].astype(np.float64)


def make_inputs(centers, radii, colors, nc_slots):
    ins = []
    for core in range(N_CORES):
        y0 = ROWS * core
        idx, cx, cy, r, col = _build_core_inputs(centers, radii, colors, core)
        n = len(idx)
        assert n <= nc_slots
        scal = np.zeros((ROWS, nc_slots * 8), np.float32)
        offs = np.zeros((1, nc_slots), np.int32)
        lhsT = np.zeros((2, nc_slots * ROWS), np.float32)
        rhs = np.zeros((2, nc_slots * WIN), np.float32)

        p = np.arange(ROWS, dtype=np.float64)
        j = np.arange(WIN, dtype=np.float64)
        for k in range(n):
            off = int(np.clip(np.floor(cx[k]) - 112.0, 0.0, float(W - WIN)))
            offs[0, k] = off
            dy2 = (y0 + p + 0.5 - cy[k]) ** 2
            dx2 = (off + j + 0.5 - cx[k]) ** 2
            alpha = col[k, 3]
            scal[:, k * 8 + 2] = 2.0 * r[k]
            scal[:, k * 8 + 3] = alpha * col[k, 0]
            scal[:, k * 8 + 4] = alpha * col[k, 1]
            scal[:, k * 8 + 5] = alpha * col[k, 2]
            scal[:, k * 8 + 6] = -alpha
            lhsT[0, k * ROWS:(k + 1) * ROWS] = dy2.astype(np.float32)
            lhsT[1, k * ROWS:(k + 1) * ROWS] = 1.0
            rhs[0, k * WIN:(k + 1) * WIN] = 1.0
            rhs[1, k * WIN:(k + 1) * WIN] = dx2.astype(np.float32)
        ins.append({"scal": scal, "offs": offs, "lhsT": lhsT, "rhs": rhs})
    return ins


def build_nc(nc_slots):
    nc = bacc.Bacc("TRN2", target_bir_lowering=False, debug=False,
                   num_devices=N_CORES)
    scal_d = nc.dram_tensor("scal", [ROWS, nc_slots * 8], F32,
                            kind="ExternalInput").ap()
    offs_d = nc.dram_tensor("offs", [1, nc_slots], mybir.dt.int32,
                            kind="ExternalInput").ap()
    lhsT_d = nc.dram_tensor("lhsT", [2, nc_slots * ROWS], F32,
                            kind="ExternalInput").ap()
    rhs_d = nc.dram_tensor("rhs", [2, nc_slots * WIN], F32,
                           kind="ExternalInput").ap()
    out_d = nc.dram_tensor("out", [ROWS, W * 4], F32,
                           kind="ExternalOutput").ap()

    n_chunks = (nc_slots + CHUNK - 1) // CHUNK

    with TileContext(nc) as tc:
        # persistent state
        T = nc.alloc_sbuf_tensor("T", [ROWS, W], F32).ap()
        CR = nc.alloc_sbuf_tensor("CR", [ROWS, W], F32).ap()
        CG = nc.alloc_sbuf_tensor("CG", [ROWS, W], F32).ap()
        CB = nc.alloc_sbuf_tensor("CB", [ROWS, W], F32).ap()
        out_sb = nc.alloc_sbuf_tensor("out_sb", [ROWS, W * 4], F32).ap()
        ch = min(CHUNK, nc_slots)
        zring = nc.alloc_sbuf_tensor("zring", [ROWS, ch * WIN], F32).ap()
        scal_sb = nc.alloc_sbuf_tensor("scal_sb", [ROWS, nc_slots * 8], F32).ap()
        offs_sb = nc.alloc_sbuf_tensor("offs_sb", [1, nc_slots],
                                       mybir.dt.int32).ap()

        nc.sync.dma_start(scal_sb, scal_d)
        nc.sync.dma_start(offs_sb, offs_d)
        nc.vector.memset(T, 1.0)
        nc.vector.memset(CR, 0.0)
        nc.vector.memset(CG, 0.0)
        nc.gpsimd.memset(CB, 0.0)

        with (
            tc.tile_pool(name="psum", bufs=2, space="PSUM") as psum_pool,
            tc.tile_pool(name="ops", bufs=2) as oppool,
            tc.tile_pool(name="d4", bufs=3) as dpool,
            tc.tile_pool(name="cov", bufs=2) as covpool,
            tc.tile_pool(name="w", bufs=4) as wpool,
            tc.tile_pool(name="tmpb", bufs=4) as bpool,
        ):
            prev_v = None
            prev_g = None
            for chunk0 in range(0, nc_slots, CHUNK):
                nk = min(CHUNK, nc_slots - chunk0)
                # ---------- phase 1: d2 (PE) -> sqrt (ACT) -> z (GPSIMD) ----
                sqrt_instrs = []
                for g8 in range(0, nk, 8):
                    gn = min(8, nk - g8)
                    lh_t = oppool.tile([2, 8 * ROWS], F32, tag="lh")
                    rh_t = oppool.tile([2, 8 * WIN], F32, tag="rh")
                    k0 = chunk0 + g8
                    nc.sync.dma_start(
                        lh_t[:, :gn * ROWS],
                        lhsT_d[:, k0 * ROWS:(k0 + gn) * ROWS])
                    nc.sync.dma_start(
                        rh_t[:, :gn * WIN],
                        rhs_d[:, k0 * WIN:(k0 + gn) * WIN])
                    for g4 in range(0, gn, 4):
                        g4n = min(4, gn - g4)
                        pt = psum_pool.tile([ROWS, 4 * 512], F32)
                        for i in range(g4n):
                            kl = g8 + g4 + i
                            nc.tensor.matmul(
                                pt[:, i * 512:i * 512 + WIN],
                                lh_t[:, (g4 + i) * ROWS:(g4 + i + 1) * ROWS],
                                rh_t[:, (g4 + i) * WIN:(g4 + i + 1) * WIN],
                                start=True, stop=True)
                        d4 = dpool.tile([ROWS, 4 * WIN], F32)
                        pview = pt.rearrange("p (b f) -> p b f", f=512)
                        dview = d4.rearrange("p (b f) -> p b f", f=WIN)
                        si = nc.scalar.activation(
                            dview[:, :g4n, :], pview[:, :g4n, :WIN], AF.Sqrt)
                        if sqrt_instrs:
                            add_dep_helper(si.ins, sqrt_instrs[-1].ins,
                                           sync=False,
                                           reason="ACT table order")
                        sqrt_instrs.append(si)
                        for i in range(g4n):
                            k = chunk0 + g8 + g4 + i   # global slot
                            nc.vector.tensor_scalar(
                                zring[:, (k - chunk0) * WIN:
                                      (k - chunk0 + 1) * WIN],
                                d4[:, i * WIN:(i + 1) * WIN],
                                -2.0,
                                scal_sb[:, k * 8 + 2:k * 8 + 3],
                                OP.mult, OP.add)

                # ---------- phase 2: sigmoid (ACT) -> composite (DVE/GPS) ---
                for g8 in range(0, nk, 8):
                    gn = min(8, nk - g8)
                    cov8 = covpool.tile([ROWS, 8 * WIN], F32)
                    sg = nc.scalar.activation(
                        cov8[:, :gn * WIN],
                        zring[:, g8 * WIN:(g8 + gn) * WIN],
                        AF.Sigmoid)
                    add_dep_helper(sg.ins, sqrt_instrs[-1].ins, sync=False,
                                   reason="sigmoid after all sqrt")
                    for i in range(gn):
                        k = chunk0 + g8 + i
                        # load the window offset; chain the load after the
                        # previous circle's last op so register live ranges
                        # stay short (the scheduler otherwise hoists all
                        # loads to the front and runs out of registers).
                        regv = nc.vector.alloc_register(f"offv_{k}")
                        liv = nc.vector.reg_load(regv, offs_sb[0:1, k:k + 1])
                        if prev_v is not None:
                            add_dep_helper(liv.ins, prev_v.ins, sync=False,
                                           reason="reg pressure")
                        off = nc.vector.snap(regv, donate=True,
                                             min_val=0, max_val=W - WIN)
                        tw = T[:, bass.ds(off, WIN)]
                        w = wpool.tile([ROWS, WIN], F32)
                        nc.vector.tensor_tensor(
                            w, tw, cov8[:, i * WIN:(i + 1) * WIN], OP.mult)
                        nc.vector.scalar_tensor_tensor(
                            tw, w, scal_sb[:, k * 8 + 6:k * 8 + 7], tw,
                            OP.mult, OP.add)
                        crw = CR[:, bass.ds(off, WIN)]
                        nc.vector.scalar_tensor_tensor(
                            crw, w, scal_sb[:, k * 8 + 3:k * 8 + 4], crw,
                            OP.mult, OP.add)
                        cgw = CG[:, bass.ds(off, WIN)]
                        prev_v = nc.vector.scalar_tensor_tensor(
                            cgw, w, scal_sb[:, k * 8 + 4:k * 8 + 5], cgw,
                            OP.mult, OP.add)
                        # B channel: ACT does the scale (Copy is in every
                        # table set), GPSIMD does the accumulate (Pool has
                        # no scalar_tensor_tensor opcode).
                        tmpb = bpool.tile([ROWS, WIN], F32)
                        nc.scalar.activation(
                            tmpb, w, AF.Copy,
                            scale=scal_sb[:, k * 8 + 5:k * 8 + 6])
                        regg = nc.gpsimd.alloc_register(f"offg_{k}")
                        lig = nc.gpsimd.reg_load(regg, offs_sb[0:1, k:k + 1])
                        if prev_g is not None:
                            add_dep_helper(lig.ins, prev_g.ins, sync=False,
                                           reason="reg pressure")
                        offg = nc.gpsimd.snap(regg, donate=True,
                                              min_val=0, max_val=W - WIN)
                        cbw = CB[:, bass.ds(offg, WIN)]
                        prev_g = nc.gpsimd.tensor_tensor(
                            cbw, cbw, tmpb, OP.add)

        # ---------- finish: interleave RGBA and store ----------
        ov = out_sb.rearrange("p (x c) -> p x c", c=4)
        nc.vector.tensor_copy(ov[:, :, 0], CR)
        nc.vector.tensor_copy(ov[:, :, 1], CG)
        nc.gpsimd.tensor_copy(ov[:, :, 2], CB)
        nc.vector.tensor_scalar(ov[:, :, 3], T, -1.0, 1.0, OP.mult, OP.add)
        nc.sync.dma_start(out_d, out_sb)

    nc.compile()
    return nc


_CACHE = {}


def _get_nc(nc_slots):
    if nc_slots not in _CACHE:
        _CACHE[nc_slots] = build_nc(nc_slots)
    return _CACHE[nc_slots]


def kernel(centers, radii, colors):
    centers = np.asarray(centers, np.float32)
    radii = np.asarray(radii, np.float32)
    colors = np.asarray(colors, np.float32)

    counts = []
    for core in range(N_CORES):
        idx, *_ = _build_core_inputs(centers, radii, colors, core)
        counts.append(len(idx))
    nc_slots = max(8, ((max(counts) + 7) // 8) * 8)

    nc = _get_nc(nc_slots)
    ins = make_inputs(centers, radii, colors, nc_slots)
    res = bass_utils.run_bass_kernel_spmd(nc, ins, list(range(N_CORES)),
                                          trace=False)
    out = np.concatenate(
        [res.results[c]["out"].reshape(ROWS, W, 4) for c in range(N_CORES)],
        axis=0)
    return out


# revision 8
# speedup vs baseline: 1.1819x; 1.1819x over previous
"""DiffVG-style circle renderer on 8 Trainium2 NeuronCores.

Strategy: shard the 1024x1024 image by rows (128 rows per core). Each core
composites only the circles whose vertical span intersects its row band
(sigmoid coverage is < 1.2e-7 beyond r+8 px of the edge), processing each
circle front-to-back on a 224-column window around its center:

    cov = sigmoid(2*r - 2*sqrt(dx^2 + dy^2))    (per pixel)
    w   = T * cov                                (T = transmittance, init 1)
    C  += w * (alpha * color);  T -= alpha * w   (premultiplied accumulation)

Final:  rgb = C, a = 1 - T  (identical to the sequential 'over' scan).

Engine split per circle:
  PE     d^2 = dy^2 (+) dx^2 outer-sum via K=4 matmul (bf16 hi/lo split
         operands for f32-level accuracy at bf16 speed)
  ACT    batched sqrt (phase 1, sqrt table) -> d ring;
         per-circle sigmoid(-2*d + 2r) (phase 2, sigmoid table);
         B-channel scale copy (Copy lives in every table set)
  DVE    w = T*cov, T-MAC, R-MAC, G-MAC on dynamic 224-px windows
  GPSIMD B-channel accumulate
Explicit dep edges keep all sqrts before all sigmoids (one table switch).
"""

import sys

if "/opt/trn_rl_repo" not in sys.path:
    sys.path.insert(0, "/opt/trn_rl_repo")

import numpy as np
import ml_dtypes

import concourse.bass as bass
import concourse.bacc as bacc
import concourse.mybir as mybir
from concourse.tile import TileContext, add_dep_helper
from concourse import bass_utils

H = 1024
W = 1024
ROWS = 128          # rows per core
N_CORES = 8
WIN = 224           # column window per circle (covers 2*(r+8) for r < 100)
MARGIN = 8.0        # sigmoid(-2*8) ~ 1.1e-7
CHUNK = 72          # max circle slots per phase pass (SBUF budget)
F32 = mybir.dt.float32
BF16 = mybir.dt.bfloat16
AF = mybir.ActivationFunctionType
OP = mybir.AluOpType
BF = ml_dtypes.bfloat16


def _build_core_inputs(centers, radii, colors, core):
    """Per-core circle list (slots ordered top-circle-first)."""
    y0 = ROWS * core
    cy = centers[:, 1].astype(np.float64)
    cx = centers[:, 0].astype(np.float64)
    r = radii.astype(np.float64)
    keep = (cy + r + MARGIN >= y0 + 0.5) & (cy - r - MARGIN <= y0 + ROWS - 0.5)
    idx = np.where(keep)[0][::-1]  # reversed: topmost (last-drawn) first
    return idx, cx[idx], cy[idx], r[idx], colors[idx].astype(np.float64)


def _hilo(x):
    hi = x.astype(BF)
    lo = (x - hi.astype(np.float64)).astype(BF)
    return hi, lo


def make_inputs(centers, radii, colors, nc_slots):
    ins = []
    for core in range(N_CORES):
        y0 = ROWS * core
        idx, cx, cy, r, col = _build_core_inputs(centers, radii, colors, core)
        n = len(idx)
        assert n <= nc_slots
        scal = np.zeros((ROWS, nc_slots * 8), np.float32)
        offs = np.zeros((1, nc_slots), np.int32)
        lhsT = np.zeros((4, nc_slots * ROWS), BF)
        rhs = np.zeros((4, nc_slots * WIN), BF)

        p = np.arange(ROWS, dtype=np.float64)
        j = np.arange(WIN, dtype=np.float64)
        for k in range(n):
            off = int(np.clip(np.floor(cx[k]) - 112.0, 0.0, float(W - WIN)))
            offs[0, k] = off
            dy2 = (y0 + p + 0.5 - cy[k]) ** 2
            dx2 = (off + j + 0.5 - cx[k]) ** 2
            alpha = col[k, 3]
            scal[:, k * 8 + 2] = 2.0 * r[k]
            scal[:, k * 8 + 3] = alpha * col[k, 0]
            scal[:, k * 8 + 4] = alpha * col[k, 1]
            scal[:, k * 8 + 5] = alpha * col[k, 2]
            scal[:, k * 8 + 6] = -alpha
            yh, yl = _hilo(dy2)
            xh, xl = _hilo(dx2)
            lhsT[0, k * ROWS:(k + 1) * ROWS] = yh
            lhsT[1, k * ROWS:(k + 1) * ROWS] = yl
            lhsT[2, k * ROWS:(k + 1) * ROWS] = 1.0
            lhsT[3, k * ROWS:(k + 1) * ROWS] = 1.0
            rhs[0, k * WIN:(k + 1) * WIN] = 1.0
            rhs[1, k * WIN:(k + 1) * WIN] = 1.0
            rhs[2, k * WIN:(k + 1) * WIN] = xh
            rhs[3, k * WIN:(k + 1) * WIN] = xl
        ins.append({"scal": scal, "offs": offs, "lhsT": lhsT, "rhs": rhs})
    return ins


def build_nc(nc_slots):
    nc = bacc.Bacc("TRN2", target_bir_lowering=False, debug=False,
                   num_devices=N_CORES)
    scal_d = nc.dram_tensor("scal", [ROWS, nc_slots * 8], F32,
                            kind="ExternalInput").ap()
    offs_d = nc.dram_tensor("offs", [1, nc_slots], mybir.dt.int32,
                            kind="ExternalInput").ap()
    lhsT_d = nc.dram_tensor("lhsT", [4, nc_slots * ROWS], BF16,
                            kind="ExternalInput").ap()
    rhs_d = nc.dram_tensor("rhs", [4, nc_slots * WIN], BF16,
                           kind="ExternalInput").ap()
    out_d = nc.dram_tensor("out", [ROWS, W * 4], F32,
                           kind="ExternalOutput").ap()

    with TileContext(nc) as tc:
        # persistent state
        T = nc.alloc_sbuf_tensor("T", [ROWS, W], F32).ap()
        CR = nc.alloc_sbuf_tensor("CR", [ROWS, W], F32).ap()
        CG = nc.alloc_sbuf_tensor("CG", [ROWS, W], F32).ap()
        CB = nc.alloc_sbuf_tensor("CB", [ROWS, W], F32).ap()
        out_sb = nc.alloc_sbuf_tensor("out_sb", [ROWS, W * 4], F32).ap()
        ch = min(CHUNK, nc_slots)
        dring = nc.alloc_sbuf_tensor("dring", [ROWS, ch * WIN], F32).ap()
        scal_sb = nc.alloc_sbuf_tensor("scal_sb", [ROWS, nc_slots * 8],
                                       F32).ap()
        offs_sb = nc.alloc_sbuf_tensor("offs_sb", [1, nc_slots],
                                       mybir.dt.int32).ap()

        nc.sync.dma_start(scal_sb, scal_d)
        nc.sync.dma_start(offs_sb, offs_d)
        nc.vector.memset(T, 1.0)
        nc.vector.memset(CR, 0.0)
        nc.vector.memset(CG, 0.0)
        nc.gpsimd.memset(CB, 0.0)

        with (
            tc.tile_pool(name="psum", bufs=2, space="PSUM") as psum_pool,
            tc.tile_pool(name="ops", bufs=2) as oppool,
            tc.tile_pool(name="cov", bufs=4) as covpool,
            tc.tile_pool(name="w", bufs=4) as wpool,
            tc.tile_pool(name="tmpb", bufs=4) as bpool,
        ):
            prev_v = None
            prev_g = None
            for chunk0 in range(0, nc_slots, CHUNK):
                nk = min(CHUNK, nc_slots - chunk0)
                # ---------- phase 1: d2 (PE) -> batched sqrt (ACT) ----------
                sqrt_instrs = []
                for g8 in range(0, nk, 8):
                    gn = min(8, nk - g8)
                    lh_t = oppool.tile([4, 8 * ROWS], BF16, tag="lh")
                    rh_t = oppool.tile([4, 8 * WIN], BF16, tag="rh")
                    k0 = chunk0 + g8
                    nc.sync.dma_start(
                        lh_t[:, :gn * ROWS],
                        lhsT_d[:, k0 * ROWS:(k0 + gn) * ROWS])
                    nc.sync.dma_start(
                        rh_t[:, :gn * WIN],
                        rhs_d[:, k0 * WIN:(k0 + gn) * WIN])
                    for g4 in range(0, gn, 4):
                        g4n = min(4, gn - g4)
                        pt = psum_pool.tile([ROWS, 4 * 512], F32)
                        for i in range(g4n):
                            nc.tensor.matmul(
                                pt[:, i * 512:i * 512 + WIN],
                                lh_t[:, (g4 + i) * ROWS:(g4 + i + 1) * ROWS],
                                rh_t[:, (g4 + i) * WIN:(g4 + i + 1) * WIN],
                                start=True, stop=True)
                        pview = pt.rearrange("p (b f) -> p b f", f=512)
                        dbase = (g8 + g4) * WIN
                        dview = dring[:, dbase:dbase + g4n * WIN].rearrange(
                            "p (b f) -> p b f", f=WIN)
                        si = nc.scalar.activation(
                            dview, pview[:, :g4n, :WIN], AF.Sqrt)
                        if sqrt_instrs:
                            add_dep_helper(si.ins, sqrt_instrs[-1].ins,
                                           sync=False,
                                           reason="ACT table order")
                        sqrt_instrs.append(si)

                # ---------- phase 2: sigmoid (ACT) -> composite (DVE/GPS) ---
                for g8 in range(0, nk, 8):
                    gn = min(8, nk - g8)
                    k0 = chunk0 + g8
                    # batched window-offset register loads (one per engine)
                    vregs = [nc.vector.alloc_register(f"offv_{k0}_{i}")
                             for i in range(gn)]
                    liv = nc.vector.reg_load(vregs, offs_sb[0:1, k0:k0 + gn])
                    if prev_v is not None:
                        add_dep_helper(liv.ins, prev_v.ins, sync=False,
                                       reason="reg pressure")
                    voff = [nc.vector.snap(rg, donate=True,
                                           min_val=0, max_val=W - WIN)
                            for rg in vregs]
                    gregs = [nc.gpsimd.alloc_register(f"offg_{k0}_{i}")
                             for i in range(gn)]
                    lig = nc.gpsimd.reg_load(gregs, offs_sb[0:1, k0:k0 + gn])
                    if prev_g is not None:
                        add_dep_helper(lig.ins, prev_g.ins, sync=False,
                                       reason="reg pressure")
                    goff = [nc.gpsimd.snap(rg, donate=True,
                                           min_val=0, max_val=W - WIN)
                            for rg in gregs]
                    for i in range(gn):
                        k = chunk0 + g8 + i
                        kl = k - chunk0
                        cov = covpool.tile([ROWS, WIN], F32)
                        sg = nc.scalar.activation(
                            cov, dring[:, kl * WIN:(kl + 1) * WIN],
                            AF.Sigmoid,
                            bias=scal_sb[:, k * 8 + 2:k * 8 + 3],
                            scale=-2.0)
                        add_dep_helper(sg.ins, sqrt_instrs[-1].ins,
                                       sync=False,
                                       reason="sigmoid after all sqrt")
                        tw = T[:, bass.ds(voff[i], WIN)]
                        w = wpool.tile([ROWS, WIN], F32)
                        nc.vector.tensor_tensor(w, tw, cov, OP.mult)
                        nc.vector.scalar_tensor_tensor(
                            tw, w, scal_sb[:, k * 8 + 6:k * 8 + 7], tw,
                            OP.mult, OP.add)
                        crw = CR[:, bass.ds(voff[i], WIN)]
                        nc.vector.scalar_tensor_tensor(
                            crw, w, scal_sb[:, k * 8 + 3:k * 8 + 4], crw,
                            OP.mult, OP.add)
                        cgw = CG[:, bass.ds(voff[i], WIN)]
                        prev_v = nc.vector.scalar_tensor_tensor(
                            cgw, w, scal_sb[:, k * 8 + 4:k * 8 + 5], cgw,
                            OP.mult, OP.add)
                        # B channel: ACT scales (Copy in every table set),
                        # GPSIMD accumulates (no stt opcode on Pool).
                        tmpb = bpool.tile([ROWS, WIN], F32)
                        nc.scalar.activation(
                            tmpb, w, AF.Copy,
                            scale=scal_sb[:, k * 8 + 5:k * 8 + 6])
                        cbw = CB[:, bass.ds(goff[i], WIN)]
                        prev_g = nc.gpsimd.tensor_tensor(
                            cbw, cbw, tmpb, OP.add)

        # ---------- finish: interleave RGBA and store ----------
        ov = out_sb.rearrange("p (x c) -> p x c", c=4)
        nc.vector.tensor_copy(ov[:, :, 0], CR)
        nc.vector.tensor_copy(ov[:, :, 1], CG)
        nc.gpsimd.tensor_copy(ov[:, :, 2], CB)
        nc.vector.tensor_scalar(ov[:, :, 3], T, -1.0, 1.0, OP.mult, OP.add)
        nc.sync.dma_start(out_d, out_sb)

    nc.compile()
    return nc


_CACHE = {}


def _get_nc(nc_slots):
    if nc_slots not in _CACHE:
        _CACHE[nc_slots] = build_nc(nc_slots)
    return _CACHE[nc_slots]


def kernel(centers, radii, colors):
    centers = np.asarray(centers, np.float32)
    radii = np.asarray(radii, np.float32)
    colors = np.asarray(colors, np.float32)

    counts = []
    for core in range(N_CORES):
        idx, *_ = _build_core_inputs(centers, radii, colors, core)
        counts.append(len(idx))
    nc_slots = max(8, ((max(counts) + 7) // 8) * 8)

    nc = _get_nc(nc_slots)
    ins = make_inputs(centers, radii, colors, nc_slots)
    res = bass_utils.run_bass_kernel_spmd(nc, ins, list(range(N_CORES)),
                                          trace=False)
    out = np.concatenate(
        [res.results[c]["out"].reshape(ROWS, W, 4) for c in range(N_CORES)],
        axis=0)
    return out
